# revision 23
# baseline (speedup 1.0000x reference)
"""Causal self-attention Trainium2 kernel v3 (8-core SPMD, tensor-parallel over heads).

B=4, T=2048, C=1024, NH=16, HS=64.  Each core owns 2 heads (128 channels).

v3 changes vs v2 (306us):
  - PV matmuls transposed: stationary = exp(S^T) chunk [128kc, 128q], moving
    = V-aug [128kc, 65] -> 65-cycle matmuls at full PE rate (vs 256-cycle
    half-width ones), and the softmax row-sums land on the q PARTITION axis.
  - Softmax normalization is now a per-partition reciprocal + one broadcast
    tensor_mul (no DRAM round-trip broadcast of 1/l).
  - y^T for the projection produced by a PE identity-matmul transpose of the
    normalized [q, d] tile (2 x 128-col matmuls per q-block).
  - Fully-masked (m1, q-half-0) quadrant skipped in S (half-width matmul)
    and PV.
  - x tile DMAs split across both DMA queues; batch-0 QKV woven with batch-0
    attention; last batch's projection chases yT q-block by q-block.
Softmax skips max-subtraction (scores ~ N(0,1)); causal masking skips
upper-triangle k-chunks and multiplies the 2 diagonal chunks by 0/1 masks
after exp.  Row-sums come from an appended ones-column in V.
"""

import numpy as np

B, T, C, NH = 4, 2048, 1024, 16
HS = C // NH            # 64
NCORES = 8
NH_LOC = NH // NCORES   # 2 heads per core
HS2 = NH_LOC * HS       # 128
TOK = B * T             # 8192
TB = T                  # tokens per batch
SCALE = 1.0 / float(np.sqrt(HS))

QB = 256                # q-block
NQB = TB // QB          # 8 q-blocks per batch
KC = 128                # k-chunk
EXPG = 2                # k-chunks per exp() call (per head)
ATTN_SLOTS = sum(qb + 2 for qb in range(NQB))   # 44
QKV_UNITS = 25          # 1 dma-prefetch unit + 4F x (q, k, v) halves
PROJ_UNITS = TB // 128  # 16

_CACHE = {}


def _build():
    import concourse.bass as bass
    import concourse.tile as tile
    from concourse import bacc, mybir

    dt = mybir.dt
    f32, bf = dt.float32, dt.bfloat16

    nc = bacc.Bacc(None, target_bir_lowering=False, debug=False)
    with tile.TileContext(nc) as tc:
        with tc.tile_pool(name="dram", bufs=1, space="DRAM") as dram:
            xT = dram.tile([C, TOK], bf, kind="ExternalInput", name="xT", uniquify=False)
            wq_d = dram.tile([128, 8, 128], bf, kind="ExternalInput", name="wq", uniquify=False)
            wk_d = dram.tile([128, 8, 128], bf, kind="ExternalInput", name="wk", uniquify=False)
            wv_d = dram.tile([128, 8, 128], bf, kind="ExternalInput", name="wv", uniquify=False)
            wp_d = dram.tile([HS2, C], bf, kind="ExternalInput", name="wp", uniquify=False)
            bq_d = dram.tile([HS2, 1], f32, kind="ExternalInput", name="bq", uniquify=False)
            bk_d = dram.tile([HS2, 1], f32, kind="ExternalInput", name="bk", uniquify=False)
            bvb_d = dram.tile([128, 2, 4, HS], f32, kind="ExternalInput", name="bvb", uniquify=False)
            m01_d = dram.tile([KC, 2, 2, QB], bf, kind="ExternalInput", name="m01", uniquify=False)
            idm_d = dram.tile([128, 128], bf, kind="ExternalInput", name="idm", uniquify=False)
            y_d = dram.tile([TOK, C], bf, kind="ExternalOutput", name="y", uniquify=False)
            _emit(nc, tc, bass, mybir, locals())
    nc.compile()
    return nc


def _emit(nc, tc, bass, mybir, io):
    dt = mybir.dt
    f32, bf = dt.float32, dt.bfloat16
    Exp = mybir.ActivationFunctionType.Exp

    xT, wq_d, wk_d, wv_d, wp_d = io["xT"], io["wq_d"], io["wk_d"], io["wv_d"], io["wp_d"]
    bq_d, bk_d, bvb_d, m01_d, idm_d, y_d = (
        io["bq_d"], io["bk_d"], io["bvb_d"], io["m01_d"], io["idm_d"], io["y_d"])

    with (
        tc.tile_pool(name="consts", bufs=1) as consts,
        tc.tile_pool(name="kpad", bufs=1) as kpadp,
        tc.tile_pool(name="xt", bufs=24) as xtp,
        tc.tile_pool(name="qt", bufs=2) as qtp,
        tc.tile_pool(name="vaug", bufs=3) as vaugp,
        tc.tile_pool(name="pt", bufs=3) as ptp,
        tc.tile_pool(name="ynorm", bufs=2) as ynp,
        tc.tile_pool(name="recs", bufs=2) as recp,
        tc.tile_pool(name="yt", bufs=2) as ytpool,
        tc.tile_pool(name="outsb", bufs=3) as outp,
        tc.tile_pool(name="mmps", bufs=2, space="PSUM") as mmps,
        tc.tile_pool(name="stps", bufs=2, space="PSUM") as stps,
        tc.tile_pool(name="pvps", bufs=2, space="PSUM") as pvps,
    ):
        # ---- constants (heavy ones DMA'd later, woven with F0/F1 x tiles
        # so the first Q/K/V matmuls aren't starved behind them) ----------
        wq_sb = consts.tile([128, 8, 128], bf, name="wq_sb")
        wk_sb = consts.tile([128, 8, 128], bf, name="wk_sb")
        wv_sb = consts.tile([128, 8, 128], bf, name="wv_sb")
        wp_sb = consts.tile([HS2, C], bf, name="wp_sb")
        bq_sb = consts.tile([HS2, 1], f32, name="bq_sb")
        bk_sb = consts.tile([HS2, 1], f32, name="bk_sb")
        bvb_sb = consts.tile([128, 2, 4, HS], f32, name="bvb_sb")
        m01_sb = consts.tile([KC, 2, 2, QB], bf, name="m01_sb")
        idm_sb = consts.tile([128, 128], bf, name="idm_sb")
        # consts split across queues by first-use time so neither queue's
        # const prefix delays the F0/F1 x tiles much.
        nc.sync.dma_start(wq_sb[:], wq_d[:])
        nc.sync.dma_start(wk_sb[:], wk_d[:])
        nc.sync.dma_start(m01_sb[:], m01_d[:])
        nc.gpsimd.dma_start(wv_sb[:], wv_d[:])
        nc.gpsimd.dma_start(bq_sb[:], bq_d[:])
        nc.gpsimd.dma_start(bk_sb[:], bk_d[:])
        nc.gpsimd.dma_start(bvb_sb[:], bvb_d[:])
        nc.gpsimd.dma_start(idm_sb[:], idm_d[:])
        nc.gpsimd.dma_start(wp_sb[:], wp_d[:])

        # K^T for both heads in one tile: partitions 0:64 = head0 dims,
        # 64:128 = head1 dims.  Double-buffered by batch parity.
        kpad = [kpadp.tile([128, TB], bf, name=f"kpad{p}") for p in range(2)]
        # PV work pieces (closures) carried across q-blocks AND batch seams
        pvwork = []

        def gen_qkv(b, st):
            base = b * TB

            def load_F(F):
                cols = bass.ds(base + F * 512, 512)
                tiles = []
                for cc in range(8):
                    xt = xtp.tile([128, 512], bf, name="xt")
                    eng = nc.sync if cc % 2 == 0 else nc.gpsimd
                    eng.dma_start(xt[:], xT[cc * 128:(cc + 1) * 128, cols])
                    tiles.append(xt)
                return tiles

            kp = kpad[b % 2]
            qT = qtp.tile([128, TB], bf, name="qT")
            st["qT"] = qT
            va = vaugp.tile([128, 2, TB // KC, HS + 2], bf, name="va")
            st["va"] = va
            nc.vector.memset(va[:, :, :, HS:HS + 1], 1.0)
            xts_cur = load_F(0)
            yield                # unit 0: F0 x-tile DMAs in flight
            for F in range(4):
                xts = xts_cur
                if F + 1 < 4:
                    xts_cur = load_F(F + 1)   # prefetch next F during Q unit
                lcols = bass.ds(F * 512, 512)
                # Q/K/V each split into 2 emission units so the weave can
                # interleave attention work at finer granularity.
                ps_q = mmps.tile([128, 512], f32, name="mm", tag="mm")
                for cc in range(4):
                    nc.tensor.matmul(ps_q[:], wq_sb[:, cc, :], xts[cc][:],
                                     start=(cc == 0), stop=False)
                yield
                for cc in range(4, 8):
                    nc.tensor.matmul(ps_q[:], wq_sb[:, cc, :], xts[cc][:],
                                     start=False, stop=(cc == 7))
                nc.vector.tensor_scalar_add(qT[:, lcols], ps_q[:], bq_sb[:])
                yield
                ps_k = mmps.tile([128, 512], f32, name="mm", tag="mm")
                for cc in range(4):
                    nc.tensor.matmul(ps_k[:], wk_sb[:, cc, :], xts[cc][:],
                                     start=(cc == 0), stop=False)
                yield
                for cc in range(4, 8):
                    nc.tensor.matmul(ps_k[:], wk_sb[:, cc, :], xts[cc][:],
                                     start=False, stop=(cc == 7))
                nc.vector.tensor_scalar_add(kp[:, lcols], ps_k[:], bk_sb[:])
                yield
                # V directly in [token, dims] layout: x-chunk as stationary.
                psv = mmps.tile([128, 4, 2, HS], f32, name="mm", tag="mm")
                for u in range(2):
                    tcu = bass.ds(u * 128, 128)
                    for cc in range(8):
                        nc.tensor.matmul(psv[:, u, :, :], xts[cc][:, tcu],
                                         wv_sb[:, cc, :],
                                         start=(cc == 0), stop=(cc == 7))
                yield
                for u in range(2, 4):
                    tcu = bass.ds(u * 128, 128)
                    for cc in range(8):
                        nc.tensor.matmul(psv[:, u, :, :], xts[cc][:, tcu],
                                         wv_sb[:, cc, :],
                                         start=(cc == 0), stop=(cc == 7))
                for h in range(2):
                    nc.vector.tensor_add(va[:, h, F * 4:(F + 1) * 4, 0:HS],
                                         psv[:, :, h, :], bvb_sb[:, h, :, :])
                yield

        def gen_attn(b, st):
            qT = st["qT"]
            va = st["va"]
            kp = kpad[b % 2]
            yT = ytpool.tile([HS2, TB], bf, name="yT")
            st["yT"] = yT
            # PV of q-block qb runs as 5 work pieces woven into qb+1's slots
            # (carried across the batch seam: the previous batch's last
            # q-block drains during this batch's ACT-light first q-blocks).
            # PSUM accumulation chains must NOT interleave within a bank, so
            # each (qh, h) chain runs start-to-stop contiguously.

            def mk_chain(pvp, pT, qh, h, nch):
                def run():
                    stop_j = nch - 2 if qh == 0 else nch - 1
                    for j in range(0, stop_j + 1):
                        nc.tensor.matmul(
                            pvp[:, qh, h, 0:HS + 1],
                            pT[:, h, j, qh * 128:(qh + 1) * 128],
                            va[:, h, j, 0:HS + 1],
                            start=(j == 0), stop=(j == stop_j))
                return run

            def mk_last(pvp, pT, nch, store):
                def run():
                    mk_chain(pvp, pT, 1, 1, nch)()
                    rec = recp.tile([128, 2, 2, 1], f32, name="rec")
                    nc.vector.reciprocal(rec[:], pvp[:, :, :, HS:HS + 1])
                    ynorm = ynp.tile([128, 2, 2, HS], bf, name="ynorm")
                    nc.vector.tensor_mul(ynorm[:], pvp[:, :, :, 0:HS],
                                         rec[:].broadcast_to([128, 2, 2, HS]))
                    store[0] = ynorm
                return run

            def mk_T(qb, store):
                def run():
                    tp = pvps.tile([128, 256], f32, name="pvp", tag="pvp")
                    for qh in range(2):
                        nc.tensor.matmul(tp[:, qh * 128:(qh + 1) * 128],
                                         store[0][:, qh, :, :], idm_sb[:],
                                         start=True, stop=True)
                    nc.vector.tensor_copy(yT[:, bass.ds(qb * QB, QB)], tp[:])
                return run

            qbs = range(NQB) if b < B - 1 else range(NQB - 1, -1, -1)
            for qb in qbs:
                nch = 2 * qb + 2
                ngroups = qb + 1
                qcols = bass.ds(qb * QB, QB)
                qcols1 = bass.ds(qb * QB + 128, 128)
                pT = ptp.tile([128, 2, 16, QB], bf, name="pT", tag="pT")
                pvp = pvps.tile([128, 2, 2, HS + 1], f32, name="pvp", tag="pvp")
                sidx = 0
                for g in range(0, nch, EXPG):
                    ge = min(g + EXPG, nch)
                    # Drain PV pieces BEFORE the S-pair: S may block at the
                    # PE queue head on its stps slot (recycled at ACT's exp
                    # pace), and the in-order queue would stall the pieces.
                    slots_left = ngroups + 1 - sidx
                    k = -(-len(pvwork) // slots_left)
                    for _ in range(k):
                        pvwork.pop(0)()
                    stp = stps.tile([128, 2, EXPG, QB], f32, name="stp", tag="stp")
                    for j in range(g, ge):
                        kc = bass.ds(j * KC, KC)
                        if j == nch - 1 and not (b == 0 and qb == 0):
                            # m1 diag chunk: q-half-0 fully masked; narrow S.
                            nc.tensor.matmul(stp[:, 0, j - g, 128:], kp[0:64, kc],
                                             qT[0:64, qcols1], start=True, stop=True)
                            nc.tensor.matmul(stp[:, 1, j - g, 128:], kp[64:128, kc],
                                             qT[64:128, qcols1], start=True, stop=True)
                        else:
                            nc.tensor.matmul(stp[:, 0, j - g, :], kp[0:64, kc],
                                             qT[0:64, qcols], start=True, stop=True)
                            nc.tensor.matmul(stp[:, 1, j - g, :], kp[64:128, kc],
                                             qT[64:128, qcols], start=True, stop=True)
                    nc.scalar.activation(pT[:, :, g:ge, :], stp[:, :, 0:ge - g, :],
                                         Exp, scale=SCALE)
                    if ge == nch:
                        # mask the two diagonal chunks (after exp: multiplicative)
                        nc.vector.tensor_mul(pT[:, :, nch - 2:nch, :],
                                             pT[:, :, nch - 2:nch, :], m01_sb[:])
                    sidx += 1
                    yield
                # ---- q-block tail slot ----
                while pvwork:
                    pvwork.pop(0)()
                store = [None]
                pvwork.extend([mk_chain(pvp, pT, 0, 0, nch),
                               mk_chain(pvp, pT, 0, 1, nch),
                               mk_chain(pvp, pT, 1, 0, nch),
                               mk_last(pvp, pT, nch, store),
                               mk_T(qb, store)])
                if b == B - 1 and qb == 0:
                    while pvwork:
                        pvwork.pop(0)()
                yield

        def gen_proj(b, st, order=None, tail_set=()):
            yT = st["yT"]
            base = b * TB
            for i in (order if order is not None else range(TB // 128)):
                osb = outp.tile([128, C], bf, name="osb")
                for nb in range(2):
                    pp = mmps.tile([128, 512], f32, name="mm", tag="mm")
                    nc.tensor.matmul(pp[:], yT[:, i * 128:(i + 1) * 128],
                                     wp_sb[:, nb * 512:(nb + 1) * 512],
                                     start=True, stop=True)
                    # in the kernel tail (ACT idle) alternate evictions across
                    # both engines; mid-kernel keep ACT free for the exp stream.
                    if i in tail_set and nb == 1:
                        nc.scalar.copy(osb[:, nb * 512:(nb + 1) * 512], pp[:])
                    else:
                        nc.vector.tensor_copy(osb[:, nb * 512:(nb + 1) * 512], pp[:])
                oeng = nc.sync if i % 2 == 1 else nc.gpsimd
                oeng.dma_start(y_d[base + i * 128:base + (i + 1) * 128, :], osb[:])
                yield

        # Interleaved software pipeline: attention of batch b (ACT-exp heavy)
        # is woven unit-by-unit with QKV(b+1) and proj(b-1) (PE heavy).
        DESC_ORDER = [14, 15, 12, 13, 10, 11, 8, 9, 6, 7, 4, 5, 2, 3, 0, 1]
        states = {0: {}}
        g0 = gen_qkv(0, states[0])
        for _ in range(7):
            next(g0)             # F0 dmas + F0 q/k/v (F1 dmas prefetched)
        # remaining 9 qkv(0) units woven into attn(0) ahead of their q-blocks
        qkv0_sched = {1: 2, 2: 2, 3: 2, 6: 2, 7: 2, 8: 2, 11: 2, 12: 2, 13: 2}
        for b in range(B):
            others = []
            if b + 1 < B:
                states[b + 1] = {}
                others.append(gen_qkv(b + 1, states[b + 1]))
            if b - 1 >= 0:
                others.append(gen_proj(b - 1, states[b - 1]))
            rr = list(others)
            slots = ATTN_SLOTS
            nunits = (QKV_UNITS if b + 1 < B else 0) + (PROJ_UNITS if b >= 1 else 0)
            emitted = 0
            slot = 0
            proj_self = None
            self_emitted = 0
            for _ in gen_attn(b, states[b]):
                slot += 1
                if b == 0:
                    for _ in range(qkv0_sched.get(slot, 0)):
                        next(g0, None)
                if b == B - 1 and self_emitted < 14:
                    # last batch runs q-blocks descending; its proj chases
                    # yT in the same order (at most 2 units per slot).
                    AVAIL = (14, 14, 22, 22, 29, 29, 35, 35, 39, 39, 42, 42, 44, 44)
                    burst = 0
                    while (self_emitted < 14 and burst < 2
                           and slot >= AVAIL[self_emitted]):
                        if proj_self is None:
                            proj_self = gen_proj(b, states[b], order=DESC_ORDER,
                                                 tail_set={0, 1, 2, 3})
                        next(proj_self)
                        self_emitted += 1
                        burst += 1
                # +4 phantom slots: hold back ~2 filler units so the batch
                # seam still has PE work queued.
                want = (slot * nunits + slots + 3) // (slots + 4)
                fburst = 0
                while emitted < want and rr and fburst < 2:
                    g = rr.pop(0)
                    try:
                        next(g)
                        rr.append(g)
                        emitted += 1
                        fburst += 1
                    except StopIteration:
                        pass
            for g in rr:
                for _ in g:
                    pass
            if b == B - 1:
                if proj_self is None:
                    proj_self = gen_proj(b, states[b], order=DESC_ORDER,
                                         tail_set={0, 1, 2, 3})
                for _ in proj_self:
                    pass


def _get_nc():
    if "nc" not in _CACHE:
        _CACHE["nc"] = _build()
    return _CACHE["nc"]


def make_in_maps(x, w_attn, b_attn, w_proj, b_proj):
    import ml_dtypes
    bf16 = ml_dtypes.bfloat16

    x = np.asarray(x, dtype=np.float32)
    w_attn = np.asarray(w_attn, dtype=np.float32)
    b_attn = np.asarray(b_attn, dtype=np.float32)
    w_proj = np.asarray(w_proj, dtype=np.float32)

    xTh = np.ascontiguousarray(x.reshape(TOK, C).T.astype(bf16))
    r = np.arange(KC)[:, None]
    s = np.arange(QB)[None, :]
    m0 = (r <= s)
    m1 = (r + KC <= s)
    m01 = np.ascontiguousarray(
        np.broadcast_to(np.stack([m0, m1], axis=1)[:, None, :, :],
                        (KC, 2, 2, QB)).astype(bf16))
    idm = np.ascontiguousarray(np.eye(128).astype(bf16))

    def rearr(w):
        # [C, 128] -> [p, cc, m] so the on-device SBUF copy is contiguous
        return np.ascontiguousarray(
            w.reshape(8, 128, HS2).transpose(1, 0, 2).astype(bf16))

    in_maps = []
    for c in range(NCORES):
        hc = slice(c * HS2, (c + 1) * HS2)
        bv = np.ascontiguousarray(b_attn[2 * C + c * HS2:2 * C + (c + 1) * HS2])
        # bvb[p, h, u, d] = bv[h*64 + d], broadcast over p and u
        bvb = np.ascontiguousarray(
            np.broadcast_to(bv.reshape(1, 2, 1, HS), (128, 2, 4, HS)).astype(np.float32))
        in_maps.append({
            "xT": xTh,
            "wq": rearr(w_attn[:, hc]),
            "wk": rearr(w_attn[:, C + c * HS2:C + (c + 1) * HS2]),
            "wv": rearr(w_attn[:, 2 * C + c * HS2:2 * C + (c + 1) * HS2]),
            "wp": np.ascontiguousarray(w_proj[hc, :].astype(bf16)),
            "bq": np.ascontiguousarray(b_attn[hc]).reshape(HS2, 1),
            "bk": np.ascontiguousarray(b_attn[C + c * HS2:C + (c + 1) * HS2]).reshape(HS2, 1),
            "bvb": bvb,
            "m01": m01,
            "idm": idm,
        })
    return in_maps


def kernel(x, w_attn, b_attn, w_proj, b_proj):
    from concourse.bass_utils import run_bass_kernel_spmd

    b_proj = np.asarray(b_proj, dtype=np.float32)
    in_maps = make_in_maps(x, w_attn, b_attn, w_proj, b_proj)
    nc = _get_nc()
    res = run_bass_kernel_spmd(nc, in_maps, core_ids=list(range(NCORES)))
    y = res.results[0]["y"].astype(np.float32)
    for c in range(1, NCORES):
        y += res.results[c]["y"].astype(np.float32)
    y += b_proj[None, :]
    return y.reshape(B, T, C)


# revision 24
# speedup vs baseline: 1.0261x; 1.0261x over previous
"""Causal self-attention Trainium2 kernel v3 (8-core SPMD, tensor-parallel over heads).

B=4, T=2048, C=1024, NH=16, HS=64.  Each core owns 2 heads (128 channels).

v3 changes vs v2 (306us):
  - PV matmuls transposed: stationary = exp(S^T) chunk [128kc, 128q], moving
    = V-aug [128kc, 65] -> 65-cycle matmuls at full PE rate (vs 256-cycle
    half-width ones), and the softmax row-sums land on the q PARTITION axis.
  - Softmax normalization is now a per-partition reciprocal + one broadcast
    tensor_mul (no DRAM round-trip broadcast of 1/l).
  - y^T for the projection produced by a PE identity-matmul transpose of the
    normalized [q, d] tile (2 x 128-col matmuls per q-block).
  - Fully-masked (m1, q-half-0) quadrant skipped in S (half-width matmul)
    and PV.
  - x tile DMAs split across both DMA queues; batch-0 QKV woven with batch-0
    attention; last batch's projection chases yT q-block by q-block.
Softmax skips max-subtraction (scores ~ N(0,1)); causal masking skips
upper-triangle k-chunks and multiplies the 2 diagonal chunks by 0/1 masks
after exp.  Row-sums come from an appended ones-column in V.
"""

import numpy as np

B, T, C, NH = 4, 2048, 1024, 16
HS = C // NH            # 64
NCORES = 8
NH_LOC = NH // NCORES   # 2 heads per core
HS2 = NH_LOC * HS       # 128
TOK = B * T             # 8192
TB = T                  # tokens per batch
SCALE = 1.0 / float(np.sqrt(HS))

QB = 256                # q-block
NQB = TB // QB          # 8 q-blocks per batch
KC = 128                # k-chunk
EXPG = 2                # k-chunks per exp() call (per head)
ATTN_SLOTS = sum(qb + 2 for qb in range(NQB))   # 44
QKV_UNITS = 25          # 1 dma-prefetch unit + 4F x (q, k, v) halves
PROJ_UNITS = TB // 128  # 16

_CACHE = {}


def _build():
    import concourse.bass as bass
    import concourse.tile as tile
    from concourse import bacc, mybir

    dt = mybir.dt
    f32, bf = dt.float32, dt.bfloat16

    nc = bacc.Bacc(None, target_bir_lowering=False, debug=False)
    with tile.TileContext(nc) as tc:
        with tc.tile_pool(name="dram", bufs=1, space="DRAM") as dram:
            xT = dram.tile([C, TOK], bf, kind="ExternalInput", name="xT", uniquify=False)
            wq_d = dram.tile([128, 8, 128], bf, kind="ExternalInput", name="wq", uniquify=False)
            wk_d = dram.tile([128, 8, 128], bf, kind="ExternalInput", name="wk", uniquify=False)
            wv_d = dram.tile([128, 8, 128], bf, kind="ExternalInput", name="wv", uniquify=False)
            wp_d = dram.tile([HS2, C], bf, kind="ExternalInput", name="wp", uniquify=False)
            bq_d = dram.tile([HS2, 1], f32, kind="ExternalInput", name="bq", uniquify=False)
            bk_d = dram.tile([HS2, 1], f32, kind="ExternalInput", name="bk", uniquify=False)
            bvb_d = dram.tile([128, 2, 4, HS], f32, kind="ExternalInput", name="bvb", uniquify=False)
            m01_d = dram.tile([KC, 2, 2, QB], bf, kind="ExternalInput", name="m01", uniquify=False)
            idm_d = dram.tile([128, 128], bf, kind="ExternalInput", name="idm", uniquify=False)
            y_d = dram.tile([TOK, C], bf, kind="ExternalOutput", name="y", uniquify=False)
            _emit(nc, tc, bass, mybir, locals())
    nc.compile()
    return nc


def _emit(nc, tc, bass, mybir, io):
    dt = mybir.dt
    f32, bf = dt.float32, dt.bfloat16
    Exp = mybir.ActivationFunctionType.Exp

    xT, wq_d, wk_d, wv_d, wp_d = io["xT"], io["wq_d"], io["wk_d"], io["wv_d"], io["wp_d"]
    bq_d, bk_d, bvb_d, m01_d, idm_d, y_d = (
        io["bq_d"], io["bk_d"], io["bvb_d"], io["m01_d"], io["idm_d"], io["y_d"])

    with (
        tc.tile_pool(name="consts", bufs=1) as consts,
        tc.tile_pool(name="kpad", bufs=1) as kpadp,
        tc.tile_pool(name="xt", bufs=24) as xtp,
        tc.tile_pool(name="qt", bufs=2) as qtp,
        tc.tile_pool(name="vaug", bufs=3) as vaugp,
        tc.tile_pool(name="pt", bufs=3) as ptp,
        tc.tile_pool(name="ynorm", bufs=2) as ynp,
        tc.tile_pool(name="recs", bufs=2) as recp,
        tc.tile_pool(name="yt", bufs=2) as ytpool,
        tc.tile_pool(name="outsb", bufs=3) as outp,
        tc.tile_pool(name="mmps", bufs=2, space="PSUM") as mmps,
        tc.tile_pool(name="stps", bufs=2, space="PSUM") as stps,
        tc.tile_pool(name="pvps", bufs=2, space="PSUM") as pvps,
    ):
        # ---- constants (heavy ones DMA'd later, woven with F0/F1 x tiles
        # so the first Q/K/V matmuls aren't starved behind them) ----------
        wq_sb = consts.tile([128, 8, 128], bf, name="wq_sb")
        wk_sb = consts.tile([128, 8, 128], bf, name="wk_sb")
        wv_sb = consts.tile([128, 8, 128], bf, name="wv_sb")
        wp_sb = consts.tile([HS2, C], bf, name="wp_sb")
        bq_sb = consts.tile([HS2, 1], f32, name="bq_sb")
        bk_sb = consts.tile([HS2, 1], f32, name="bk_sb")
        bvb_sb = consts.tile([128, 2, 4, HS], f32, name="bvb_sb")
        m01_sb = consts.tile([KC, 2, 2, QB], bf, name="m01_sb")
        idm_sb = consts.tile([128, 128], bf, name="idm_sb")
        # consts split across queues by first-use time so neither queue's
        # const prefix delays the F0/F1 x tiles much.
        nc.sync.dma_start(wq_sb[:], wq_d[:])
        nc.sync.dma_start(wk_sb[:], wk_d[:])
        nc.sync.dma_start(m01_sb[:], m01_d[:])
        nc.gpsimd.dma_start(wv_sb[:], wv_d[:])
        nc.gpsimd.dma_start(bq_sb[:], bq_d[:])
        nc.gpsimd.dma_start(bk_sb[:], bk_d[:])
        nc.gpsimd.dma_start(bvb_sb[:], bvb_d[:])
        nc.gpsimd.dma_start(idm_sb[:], idm_d[:])
        nc.gpsimd.dma_start(wp_sb[:], wp_d[:])

        # K^T for both heads in one tile: partitions 0:64 = head0 dims,
        # 64:128 = head1 dims.  Double-buffered by batch parity.
        kpad = [kpadp.tile([128, TB], bf, name=f"kpad{p}") for p in range(2)]
        # PV work pieces (closures) carried across q-blocks AND batch seams
        pvwork = []

        def gen_qkv(b, st):
            base = b * TB

            def load_F(F):
                cols = bass.ds(base + F * 512, 512)
                tiles = []
                for cc in range(8):
                    xt = xtp.tile([128, 512], bf, name="xt")
                    eng = nc.sync if cc % 2 == 0 else nc.gpsimd
                    eng.dma_start(xt[:], xT[cc * 128:(cc + 1) * 128, cols])
                    tiles.append(xt)
                return tiles

            kp = kpad[b % 2]
            qT = qtp.tile([128, TB], bf, name="qT")
            st["qT"] = qT
            va = vaugp.tile([128, 2, TB // KC, HS + 2], bf, name="va")
            st["va"] = va
            nc.vector.memset(va[:, :, :, HS:HS + 1], 1.0)
            xts_cur = load_F(0)
            yield                # unit 0: F0 x-tile DMAs in flight
            for F in range(4):
                xts = xts_cur
                if F + 1 < 4:
                    xts_cur = load_F(F + 1)   # prefetch next F during Q unit
                lcols = bass.ds(F * 512, 512)
                # Q/K/V each split into 2 emission units so the weave can
                # interleave attention work at finer granularity.
                ps_q = mmps.tile([128, 512], f32, name="mm", tag="mm")
                for cc in range(4):
                    nc.tensor.matmul(ps_q[:], wq_sb[:, cc, :], xts[cc][:],
                                     start=(cc == 0), stop=False)
                yield
                for cc in range(4, 8):
                    nc.tensor.matmul(ps_q[:], wq_sb[:, cc, :], xts[cc][:],
                                     start=False, stop=(cc == 7))
                nc.vector.tensor_scalar_add(qT[:, lcols], ps_q[:], bq_sb[:])
                yield
                ps_k = mmps.tile([128, 512], f32, name="mm", tag="mm")
                for cc in range(4):
                    nc.tensor.matmul(ps_k[:], wk_sb[:, cc, :], xts[cc][:],
                                     start=(cc == 0), stop=False)
                yield
                for cc in range(4, 8):
                    nc.tensor.matmul(ps_k[:], wk_sb[:, cc, :], xts[cc][:],
                                     start=False, stop=(cc == 7))
                nc.vector.tensor_scalar_add(kp[:, lcols], ps_k[:], bk_sb[:])
                yield
                # V directly in [token, dims] layout: x-chunk as stationary.
                psv = mmps.tile([128, 4, 2, HS], f32, name="mm", tag="mm")
                for u in range(2):
                    tcu = bass.ds(u * 128, 128)
                    for cc in range(8):
                        nc.tensor.matmul(psv[:, u, :, :], xts[cc][:, tcu],
                                         wv_sb[:, cc, :],
                                         start=(cc == 0), stop=(cc == 7))
                yield
                for u in range(2, 4):
                    tcu = bass.ds(u * 128, 128)
                    for cc in range(8):
                        nc.tensor.matmul(psv[:, u, :, :], xts[cc][:, tcu],
                                         wv_sb[:, cc, :],
                                         start=(cc == 0), stop=(cc == 7))
                for h in range(2):
                    nc.vector.tensor_add(va[:, h, F * 4:(F + 1) * 4, 0:HS],
                                         psv[:, :, h, :], bvb_sb[:, h, :, :])
                yield

        def gen_attn(b, st):
            qT = st["qT"]
            va = st["va"]
            kp = kpad[b % 2]
            yT = ytpool.tile([HS2, TB], bf, name="yT")
            st["yT"] = yT
            # PV of q-block qb runs as 5 work pieces woven into qb+1's slots
            # (carried across the batch seam: the previous batch's last
            # q-block drains during this batch's ACT-light first q-blocks).
            # PSUM accumulation chains must NOT interleave within a bank, so
            # each (qh, h) chain runs start-to-stop contiguously.

            def mk_chain(pvp, pT, qh, h, nch):
                def run():
                    stop_j = nch - 2 if qh == 0 else nch - 1
                    for j in range(0, stop_j + 1):
                        nc.tensor.matmul(
                            pvp[:, qh, h, 0:HS + 1],
                            pT[:, h, j, qh * 128:(qh + 1) * 128],
                            va[:, h, j, 0:HS + 1],
                            start=(j == 0), stop=(j == stop_j))
                return run

            def mk_last(pvp, pT, nch, store):
                def run():
                    mk_chain(pvp, pT, 1, 1, nch)()
                    rec = recp.tile([128, 2, 2, 1], f32, name="rec")
                    nc.vector.reciprocal(rec[:], pvp[:, :, :, HS:HS + 1])
                    ynorm = ynp.tile([128, 2, 2, HS], bf, name="ynorm")
                    nc.vector.tensor_mul(ynorm[:], pvp[:, :, :, 0:HS],
                                         rec[:].broadcast_to([128, 2, 2, HS]))
                    store[0] = ynorm
                return run

            def mk_T(qb, store):
                def run():
                    tp = pvps.tile([128, 256], f32, name="pvp", tag="pvp")
                    for qh in range(2):
                        nc.tensor.matmul(tp[:, qh * 128:(qh + 1) * 128],
                                         store[0][:, qh, :, :], idm_sb[:],
                                         start=True, stop=True)
                    nc.vector.tensor_copy(yT[:, bass.ds(qb * QB, QB)], tp[:])
                return run

            for qb in range(NQB):
                nch = 2 * qb + 2
                ngroups = qb + 1
                qcols = bass.ds(qb * QB, QB)
                qcols1 = bass.ds(qb * QB + 128, 128)
                pT = ptp.tile([128, 2, 16, QB], bf, name="pT", tag="pT")
                pvp = pvps.tile([128, 2, 2, HS + 1], f32, name="pvp", tag="pvp")
                sidx = 0
                for g in range(0, nch, EXPG):
                    ge = min(g + EXPG, nch)
                    # Drain PV pieces BEFORE the S-pair: S may block at the
                    # PE queue head on its stps slot (recycled at ACT's exp
                    # pace), and the in-order queue would stall the pieces.
                    slots_left = ngroups + 1 - sidx
                    k = -(-len(pvwork) // slots_left)
                    for _ in range(k):
                        pvwork.pop(0)()
                    stp = stps.tile([128, 2, EXPG, QB], f32, name="stp", tag="stp")
                    for j in range(g, ge):
                        kc = bass.ds(j * KC, KC)
                        if j == nch - 1 and not (b == 0 and qb == 0):
                            # m1 diag chunk: q-half-0 fully masked; narrow S.
                            nc.tensor.matmul(stp[:, 0, j - g, 128:], kp[0:64, kc],
                                             qT[0:64, qcols1], start=True, stop=True)
                            nc.tensor.matmul(stp[:, 1, j - g, 128:], kp[64:128, kc],
                                             qT[64:128, qcols1], start=True, stop=True)
                        else:
                            nc.tensor.matmul(stp[:, 0, j - g, :], kp[0:64, kc],
                                             qT[0:64, qcols], start=True, stop=True)
                            nc.tensor.matmul(stp[:, 1, j - g, :], kp[64:128, kc],
                                             qT[64:128, qcols], start=True, stop=True)
                    nc.scalar.activation(pT[:, :, g:ge, :], stp[:, :, 0:ge - g, :],
                                         Exp, scale=SCALE)
                    if ge == nch:
                        # mask the two diagonal chunks (after exp: multiplicative)
                        nc.vector.tensor_mul(pT[:, :, nch - 2:nch, :],
                                             pT[:, :, nch - 2:nch, :], m01_sb[:])
                    sidx += 1
                    yield
                # ---- q-block tail slot ----
                while pvwork:
                    pvwork.pop(0)()
                store = [None]
                pvwork.extend([mk_chain(pvp, pT, 0, 0, nch),
                               mk_chain(pvp, pT, 0, 1, nch),
                               mk_chain(pvp, pT, 1, 0, nch),
                               mk_last(pvp, pT, nch, store),
                               mk_T(qb, store)])
                if b == B - 1 and qb == NQB - 1:
                    while pvwork:
                        pvwork.pop(0)()
                yield

        def gen_proj(b, st, order=None, tail_set=()):
            yT = st["yT"]
            base = b * TB
            for i in (order if order is not None else range(TB // 128)):
                osb = outp.tile([128, C], bf, name="osb")
                for nb in range(2):
                    pp = mmps.tile([128, 512], f32, name="mm", tag="mm")
                    nc.tensor.matmul(pp[:], yT[:, i * 128:(i + 1) * 128],
                                     wp_sb[:, nb * 512:(nb + 1) * 512],
                                     start=True, stop=True)
                    # in the kernel tail (ACT idle) alternate evictions across
                    # both engines; mid-kernel keep ACT free for the exp stream.
                    if i in tail_set and nb == 1:
                        nc.scalar.copy(osb[:, nb * 512:(nb + 1) * 512], pp[:])
                    else:
                        nc.vector.tensor_copy(osb[:, nb * 512:(nb + 1) * 512], pp[:])
                oeng = nc.sync if i % 2 == 1 else nc.gpsimd
                oeng.dma_start(y_d[base + i * 128:base + (i + 1) * 128, :], osb[:])
                yield

        # Interleaved software pipeline: attention of batch b (ACT-exp heavy)
        # is woven unit-by-unit with QKV(b+1) and proj(b-1) (PE heavy).
        DESC_ORDER = list(range(16))
        states = {0: {}}
        g0 = gen_qkv(0, states[0])
        for _ in range(7):
            next(g0)             # F0 dmas + F0 q/k/v (F1 dmas prefetched)
        # remaining 9 qkv(0) units woven into attn(0) ahead of their q-blocks
        qkv0_sched = {1: 2, 2: 2, 3: 2, 6: 2, 7: 2, 8: 2, 11: 2, 12: 2, 13: 2}
        for b in range(B):
            others = []
            if b + 1 < B:
                states[b + 1] = {}
                others.append(gen_qkv(b + 1, states[b + 1]))
            if b - 1 >= 0:
                others.append(gen_proj(b - 1, states[b - 1]))
            rr = list(others)
            slots = ATTN_SLOTS
            nunits = (QKV_UNITS if b + 1 < B else 0) + (PROJ_UNITS if b >= 1 else 0)
            emitted = 0
            slot = 0
            proj_self = None
            self_emitted = 0
            for _ in gen_attn(b, states[b]):
                slot += 1
                if b == 0:
                    for _ in range(qkv0_sched.get(slot, 0)):
                        next(g0, None)
                if b == B - 1 and self_emitted < 14:
                    # yT(qb) is transposed as the last PV piece drained during
                    # q-block qb+1; chase it (at most 2 units per slot).
                    AVAIL = (5, 5, 9, 9, 14, 14, 19, 19, 25, 25, 32, 32, 40, 40)
                    burst = 0
                    while (self_emitted < 14 and burst < 2
                           and slot >= AVAIL[self_emitted]):
                        if proj_self is None:
                            proj_self = gen_proj(b, states[b], order=DESC_ORDER,
                                                 tail_set={12, 13, 14, 15})
                        next(proj_self)
                        self_emitted += 1
                        burst += 1
                # +4 phantom slots: hold back ~2 filler units so the batch
                # seam still has PE work queued.
                want = (slot * nunits + slots + 3) // (slots + 4)
                fburst = 0
                while emitted < want and rr and fburst < 2:
                    g = rr.pop(0)
                    try:
                        next(g)
                        rr.append(g)
                        emitted += 1
                        fburst += 1
                    except StopIteration:
                        pass
            for g in rr:
                for _ in g:
                    pass
            if b == B - 1:
                if proj_self is None:
                    proj_self = gen_proj(b, states[b], order=DESC_ORDER,
                                         tail_set={12, 13, 14, 15})
                for _ in proj_self:
                    pass


def _get_nc():
    if "nc" not in _CACHE:
        _CACHE["nc"] = _build()
    return _CACHE["nc"]


def make_in_maps(x, w_attn, b_attn, w_proj, b_proj):
    import ml_dtypes
    bf16 = ml_dtypes.bfloat16

    x = np.asarray(x, dtype=np.float32)
    w_attn = np.asarray(w_attn, dtype=np.float32)
    b_attn = np.asarray(b_attn, dtype=np.float32)
    w_proj = np.asarray(w_proj, dtype=np.float32)

    xTh = np.ascontiguousarray(x.reshape(TOK, C).T.astype(bf16))
    r = np.arange(KC)[:, None]
    s = np.arange(QB)[None, :]
    m0 = (r <= s)
    m1 = (r + KC <= s)
    m01 = np.ascontiguousarray(
        np.broadcast_to(np.stack([m0, m1], axis=1)[:, None, :, :],
                        (KC, 2, 2, QB)).astype(bf16))
    idm = np.ascontiguousarray(np.eye(128).astype(bf16))

    def rearr(w):
        # [C, 128] -> [p, cc, m] so the on-device SBUF copy is contiguous
        return np.ascontiguousarray(
            w.reshape(8, 128, HS2).transpose(1, 0, 2).astype(bf16))

    in_maps = []
    for c in range(NCORES):
        hc = slice(c * HS2, (c + 1) * HS2)
        bv = np.ascontiguousarray(b_attn[2 * C + c * HS2:2 * C + (c + 1) * HS2])
        # bvb[p, h, u, d] = bv[h*64 + d], broadcast over p and u
        bvb = np.ascontiguousarray(
            np.broadcast_to(bv.reshape(1, 2, 1, HS), (128, 2, 4, HS)).astype(np.float32))
        in_maps.append({
            "xT": xTh,
            "wq": rearr(w_attn[:, hc]),
            "wk": rearr(w_attn[:, C + c * HS2:C + (c + 1) * HS2]),
            "wv": rearr(w_attn[:, 2 * C + c * HS2:2 * C + (c + 1) * HS2]),
            "wp": np.ascontiguousarray(w_proj[hc, :].astype(bf16)),
            "bq": np.ascontiguousarray(b_attn[hc]).reshape(HS2, 1),
            "bk": np.ascontiguousarray(b_attn[C + c * HS2:C + (c + 1) * HS2]).reshape(HS2, 1),
            "bvb": bvb,
            "m01": m01,
            "idm": idm,
        })
    return in_maps


def kernel(x, w_attn, b_attn, w_proj, b_proj):
    from concourse.bass_utils import run_bass_kernel_spmd

    b_proj = np.asarray(b_proj, dtype=np.float32)
    in_maps = make_in_maps(x, w_attn, b_attn, w_proj, b_proj)
    nc = _get_nc()
    res = run_bass_kernel_spmd(nc, in_maps, core_ids=list(range(NCORES)))
    y = res.results[0]["y"].astype(np.float32)
    for c in range(1, NCORES):
        y += res.results[c]["y"].astype(np.float32)
    y += b_proj[None, :]
    return y.reshape(B, T, C)
